# revision 19
# baseline (speedup 1.0000x reference)
"""Single-head attention (B=8, D=1024, N=2048, fp32 I/O) on 8 TRN2 NeuronCores.

Sharding: data-parallel over batch — core i computes batch element i with the
full weights replicated. No collectives needed.

Per-core math (x: [D, N] features-first, W*: [D, D]):
    scores = x^T (W_q^T W_k) x / sqrt(D)        (Gram-matrix trick: no weight
    attn   = softmax(scores, axis=-1)            transposes needed on the Q/K
    out    = (W_v x) attn                        path)
  GT = W_k^T W_q              -> matmul(lhsT=W_k, rhs=W_q)     [f, e]
  U  = GT^T x = (W_q^T W_k) x -> matmul(lhsT=GT, rhs=x)        [e, m]
  S  = x^T U                  -> matmul(lhsT=x,  rhs=U)        [n, m]
  VT = x^T W_v^T              -> matmul(lhsT=x,  rhs=W_v^T)    [n, d]
  out= VT^T attn              -> matmul(lhsT=VT, rhs=attn)     [d, m]
W_v^T comes from an XBAR DMA transpose of the bf16 W_v via a DRAM scratch
(keeps the transpose off the TensorEngine). The softmax normalizer 1/Z[n] is
folded into VT's rows (n is the contraction index of the output matmul), so
attn is stored as unnormalized exp() in bf16. Compute dtype bf16 (f32 PSUM
accumulation); measured rel_l2 vs the f32 reference ~5e-3.

All PSUM tiles share one [128, 512] single-bank tag, 8 bufs = all 8 banks, so
DMA-gated phases keep 8 accumulation groups (one per bank) issuable per
arriving input chunk and slots recycle at single-bank granularity.
"""

import numpy as np

import concourse.bacc as bacc
import concourse.mybir as mybir
import concourse.tile as tile
from concourse.bass_utils import run_bass_kernel_spmd

B, D, N = 8, 1024, 2048
P = 128
CE = D // P   # 8 chunks on the feature axis
CN = N // P   # 16 chunks on the sequence axis
K_SCALE = 1.0 / float(np.sqrt(D))

F32 = mybir.dt.float32
BF16 = mybir.dt.bfloat16


def build_nc():
    nc = bacc.Bacc("TRN2", target_bir_lowering=False, debug=False)

    x_ext = nc.dram_tensor("x", [D, N], F32, kind="ExternalInput")
    wq_ext = nc.dram_tensor("W_q", [D, D], F32, kind="ExternalInput")
    wk_ext = nc.dram_tensor("W_k", [D, D], F32, kind="ExternalInput")
    wv_ext = nc.dram_tensor("W_v", [D, D], F32, kind="ExternalInput")
    out_ext = nc.dram_tensor("out", [D, N], F32, kind="ExternalOutput")

    x_re = x_ext.ap().rearrange("(c p) n -> c p n", p=P)
    wq_re = wq_ext.ap().rearrange("(c p) e -> c p e", p=P)
    wk_re = wk_ext.ap().rearrange("(c p) e -> c p e", p=P)
    wv_re = wv_ext.ap().rearrange("(c p) e -> c p e", p=P)
    out_re = out_ext.ap().rearrange("(c p) m -> c p m", p=P)

    with tile.TileContext(nc) as tc:
        with (
            tc.tile_pool(name="const", bufs=1) as const,
            tc.tile_pool(name="stage", bufs=3) as stage,
            tc.tile_pool(name="big", bufs=21) as big,
            tc.tile_pool(name="small", bufs=4) as small,
            tc.tile_pool(name="dram", bufs=1, space="DRAM") as dram,
            tc.tile_pool(name="psum", bufs=4, space="PSUM") as psum,
        ):
            recip_z = const.tile([P, CN], F32, tag="rz")

            # All big tensors share one 8KB/partition slot tag so SBUF slots
            # recycle across phases (peak ~20 live of 21 slots).
            wq_t = [big.tile([P, 4, 1024], BF16, tag="big", name=f"wq{i}") for i in range(2)]
            wk_t = [big.tile([P, 4, 1024], BF16, tag="big", name=f"wk{i}") for i in range(2)]
            wvt_t = [big.tile([P, 4, 1024], BF16, tag="big", name=f"wvt{i}") for i in range(2)]
            gt_t = [big.tile([P, 4, 1024], BF16, tag="big", name=f"gt{i}") for i in range(2)]
            x_t = [big.tile([P, 2, N], BF16, tag="big", name=f"x{i}") for i in range(4)]
            u_t = [big.tile([P, 2, N], BF16, tag="big", name=f"u{i}") for i in range(4)]
            vt_t = [big.tile([P, 4, 1024], BF16, tag="big", name=f"vt{i}") for i in range(4)]
            attn_t = [big.tile([P, 2, N], BF16, tag="big", name=f"at{i}") for i in range(8)]

            _ps_n = [0]

            def ps_tile():
                _ps_n[0] += 1
                return psum.tile(
                    [P, 512], F32, tag="ps", bufs=8, name=f"ps{_ps_n[0]}"
                )



            # ---- load + cast W_q, W_k --------------------------------------
            # Interleaved half-chunk ([128, 512] f32) loads: GT matmuls become
            # issuable at 0.25MB arrival granularity, so the first matmul
            # starts earlier and the arrival-gated trickle has fewer holes.
            for c in range(CE):
                for w_re_, w_t in ((wq_re, wq_t), (wk_re, wk_t)):
                    for h in range(2):
                        st = stage.tile([P, 512], F32, tag="stage")
                        nc.sync.dma_start(st[:], w_re_[c][:, h * 512:(h + 1) * 512])
                        nc.vector.tensor_copy(
                            w_t[c // 4][:, c % 4, h * 512:(h + 1) * 512], st[:]
                        )

            # ---- load + cast x [e, n] --------------------------------------
            # Casts on the (otherwise idle) scalar engine: the in-order
            # vector engine must not head-of-line block the GT/U psum evicts
            # behind these DMA-gated casts.
            # x rides the scalar-engine HWDGE queue, concurrent with the
            # weight loads on the sync queue.
            for c in range(CE):
                st = stage.tile([P, N], F32, tag="stage")
                nc.scalar.dma_start(st[:], x_re[c])
                nc.scalar.copy(x_t[c // 2][:, c % 2, :], st[:])

            # ---- W_v: load, cast, XBAR-transpose via DRAM scratch ----------
            wv_scratch = dram.tile([D, D], BF16)
            for cd in range(CE):
                st = stage.tile([P, D], F32, tag="stage")
                nc.sync.dma_start(st[:], wv_re[cd])
                wvc = small.tile([P, D], BF16, tag="wvc", bufs=2)
                nc.scalar.copy(wvc[:], st[:])
                nc.sync.dma_start(wv_scratch[cd * P:(cd + 1) * P, :], wvc[:])
            for ce in range(CE):
                nc.sync.dma_start(
                    out=wvt_t[ce // 4][:, ce % 4, :],
                    in_=wv_scratch[:, ce * P:(ce + 1) * P],
                    transpose=True,
                )

            # ---- GT = W_k^T W_q  [f, e] ------------------------------------
            # Waves of 8 single-bank psum tiles with the contraction (dc)
            # loop outermost: each arriving W chunk pair feeds 8 issuable
            # matmuls, and single-bank tiles recycle finely at wave edges.
            for wave in range(2):
                cfs = range(wave * 4, wave * 4 + 4)
                tiles = {(cf, et): ps_tile() for cf in cfs for et in range(2)}
                for dc in range(CE):
                    for cf in cfs:
                        for et in range(2):
                            nc.tensor.matmul(
                                tiles[cf, et][:],
                                wk_t[dc // 4][:, dc % 4, cf * P:(cf + 1) * P],
                                wq_t[dc // 4][:, dc % 4, et * 512:(et + 1) * 512],
                                start=(dc == 0),
                                stop=(dc == CE - 1),
                            )
                for (cf, et), ps in tiles.items():
                    nc.vector.tensor_copy(
                        gt_t[cf // 4][:, cf % 4, et * 512:(et + 1) * 512], ps[:]
                    )

            # ---- U = GT^T x = (W_q^T W_k) x  [e, m] ------------------------
            for wave in range(4):
                ces = (2 * wave, 2 * wave + 1)
                tiles = {(ce, mt): ps_tile() for ce in ces for mt in range(4)}
                for cf in range(CE):
                    for ce in ces:
                        for mt in range(4):
                            nc.tensor.matmul(
                                tiles[ce, mt][:],
                                gt_t[cf // 4][:, cf % 4, ce * P:(ce + 1) * P],
                                x_t[cf // 2][:, cf % 2, mt * 512:(mt + 1) * 512],
                                start=(cf == 0),
                                stop=(cf == CE - 1),
                            )
                for (ce, mt), ps in tiles.items():
                    nc.vector.tensor_copy(
                        u_t[ce // 2][:, ce % 2, mt * 512:(mt + 1) * 512], ps[:]
                    )

            # ---- scores + softmax per 128-row chunk ------------------------
            for cn in range(CN):
                quads = [ps_tile() for _ in range(4)]
                for ce in range(CE):
                    for q in range(4):
                        nc.tensor.matmul(
                            quads[q][:],
                            x_t[ce // 2][:, ce % 2, cn * P:(cn + 1) * P],
                            u_t[ce // 2][:, ce % 2, q * 512:(q + 1) * 512],
                            start=(ce == 0),
                            stop=(ce == CE - 1),
                        )
                mx = [small.tile([P, 1], F32, tag="sm", bufs=8, name=f"mx{cn}_{q}") for q in range(4)]
                for q in range(4):
                    nc.vector.reduce_max(mx[q][:], quads[q][:], axis=mybir.AxisListType.X)
                bias = small.tile([P, 1], F32, tag="sm", bufs=8)
                nc.vector.tensor_max(mx[0][:], mx[0][:], mx[1][:])
                nc.vector.tensor_max(mx[2][:], mx[2][:], mx[3][:])
                nc.vector.tensor_max(bias[:], mx[0][:], mx[2][:])
                nc.vector.tensor_scalar_mul(bias[:], bias[:], -K_SCALE)
                zq = [small.tile([P, 1], F32, tag="sm", bufs=8, name=f"z{cn}_{q}") for q in range(4)]
                for q in range(4):
                    nc.scalar.activation(
                        attn_t[cn // 2][:, cn % 2, q * 512:(q + 1) * 512],
                        quads[q][:],
                        mybir.ActivationFunctionType.Exp,
                        bias=bias[:],
                        scale=K_SCALE,
                        accum_out=zq[q][:],
                    )
                nc.vector.tensor_add(zq[0][:], zq[0][:], zq[1][:])
                nc.vector.tensor_add(zq[2][:], zq[2][:], zq[3][:])
                nc.vector.tensor_add(zq[0][:], zq[0][:], zq[2][:])
                nc.vector.reciprocal(recip_z[:, cn:cn + 1], zq[0][:])

            # ---- VT = x^T WvT  [n, d] --------------------------------------
            # After scores in the PE stream: WvT's DMA transpose arrives late
            # in the load queue, and nothing before AV needs VT.
            for cn in range(CN):
                vts = [ps_tile() for _ in range(2)]
                for ce in range(CE):
                    for dt in range(2):
                        nc.tensor.matmul(
                            vts[dt][:],
                            x_t[ce // 2][:, ce % 2, cn * P:(cn + 1) * P],
                            wvt_t[ce // 4][:, ce % 4, dt * 512:(dt + 1) * 512],
                            start=(ce == 0),
                            stop=(ce == CE - 1),
                        )
                for dt in range(2):
                    nc.vector.tensor_copy(
                        vt_t[cn // 4][:, cn % 4, dt * 512:(dt + 1) * 512], vts[dt][:]
                    )

            # ---- fold 1/Z into VT rows -------------------------------------
            for cn in range(CN):
                nc.vector.tensor_scalar_mul(
                    vt_t[cn // 4][:, cn % 4, :],
                    vt_t[cn // 4][:, cn % 4, :],
                    recip_z[:, cn:cn + 1],
                )

            # ---- out = VTs^T attn  [d, m] ----------------------------------
            for dt in range(CE):
                ot = stage.tile([P, N], F32, tag="stage")
                outs = [ps_tile() for _ in range(4)]
                for cn in range(CN):
                    for q in range(4):
                        nc.tensor.matmul(
                            outs[q][:],
                            vt_t[cn // 4][:, cn % 4, dt * P:(dt + 1) * P],
                            attn_t[cn // 2][:, cn % 2, q * 512:(q + 1) * 512],
                            start=(cn == 0),
                            stop=(cn == CN - 1),
                        )
                for q in range(4):
                    nc.vector.tensor_copy(ot[:, q * 512:(q + 1) * 512], outs[q][:])
                    if q % 2 == 1:
                        nc.sync.dma_start(
                            out_re[dt][:, (q - 1) * 512:(q + 1) * 512],
                            ot[:, (q - 1) * 512:(q + 1) * 512],
                        )

    nc.compile()
    return nc


_NC = None


def _get_nc():
    global _NC
    if _NC is None:
        _NC = build_nc()
    return _NC


def make_in_maps(x, W_q, W_k, W_v):
    return [
        {"x": x[i], "W_q": W_q, "W_k": W_k, "W_v": W_v} for i in range(B)
    ]


def kernel(x, W_q, W_k, W_v):
    x = np.ascontiguousarray(np.asarray(x, dtype=np.float32))
    W_q = np.ascontiguousarray(np.asarray(W_q, dtype=np.float32))
    W_k = np.ascontiguousarray(np.asarray(W_k, dtype=np.float32))
    W_v = np.ascontiguousarray(np.asarray(W_v, dtype=np.float32))
    assert x.shape == (B, D, N), x.shape

    nc = _get_nc()
    res = run_bass_kernel_spmd(
        nc, make_in_maps(x, W_q, W_k, W_v), core_ids=list(range(B))
    )
    return np.stack([res.results[i]["out"] for i in range(B)], axis=0)


if __name__ == "__main__":
    rng = np.random.default_rng(0)
    scale = 1.0 / np.sqrt(D)
    x = rng.standard_normal((B, D, N), dtype=np.float32)
    wq = rng.standard_normal((D, D), dtype=np.float32) * scale
    wk = rng.standard_normal((D, D), dtype=np.float32) * scale
    wv = rng.standard_normal((D, D), dtype=np.float32) * scale
    out = kernel(x, wq, wk, wv)
    print("out", out.shape, out.dtype, np.abs(out).max())


# revision 20
# speedup vs baseline: 1.0439x; 1.0439x over previous
"""Single-head attention (B=8, D=1024, N=2048, fp32 I/O) on 8 TRN2 NeuronCores.

Sharding: data-parallel over batch — core i computes batch element i with the
full weights replicated. No collectives needed.

Per-core math (x: [D, N] features-first, W*: [D, D]):
    scores = x^T (W_q^T W_k) x / sqrt(D)        (Gram-matrix trick: no weight
    attn   = softmax(scores, axis=-1)            transposes needed on the Q/K
    out    = (W_v x) attn                        path)
  GT = W_k^T W_q              -> matmul(lhsT=W_k, rhs=W_q)     [f, e]
  U  = GT^T x = (W_q^T W_k) x -> matmul(lhsT=GT, rhs=x)        [e, m]
  S  = x^T U                  -> matmul(lhsT=x,  rhs=U)        [n, m]
  VT = x^T W_v^T              -> matmul(lhsT=x,  rhs=W_v^T)    [n, d]
  out= VT^T attn              -> matmul(lhsT=VT, rhs=attn)     [d, m]
W_v^T comes from an XBAR DMA transpose of the bf16 W_v via a DRAM scratch
(keeps the transpose off the TensorEngine). The softmax normalizer 1/Z[n] is
folded into VT's rows (n is the contraction index of the output matmul), so
attn is stored as unnormalized exp() in bf16. Compute dtype bf16 (f32 PSUM
accumulation); measured rel_l2 vs the f32 reference ~5e-3.

All PSUM tiles share one [128, 512] single-bank tag, 8 bufs = all 8 banks, so
DMA-gated phases keep 8 accumulation groups (one per bank) issuable per
arriving input chunk and slots recycle at single-bank granularity.
"""

import numpy as np

import concourse.bacc as bacc
import concourse.mybir as mybir
import concourse.tile as tile
from concourse.bass_utils import run_bass_kernel_spmd

B, D, N = 8, 1024, 2048
P = 128
CE = D // P   # 8 chunks on the feature axis
CN = N // P   # 16 chunks on the sequence axis
K_SCALE = 1.0 / float(np.sqrt(D))

F32 = mybir.dt.float32
BF16 = mybir.dt.bfloat16


def build_nc():
    nc = bacc.Bacc("TRN2", target_bir_lowering=False, debug=False)

    x_ext = nc.dram_tensor("x", [D, N], F32, kind="ExternalInput")
    wq_ext = nc.dram_tensor("W_q", [D, D], F32, kind="ExternalInput")
    wk_ext = nc.dram_tensor("W_k", [D, D], F32, kind="ExternalInput")
    wv_ext = nc.dram_tensor("W_v", [D, D], F32, kind="ExternalInput")
    out_ext = nc.dram_tensor("out", [D, N], F32, kind="ExternalOutput")

    x_re = x_ext.ap().rearrange("(c p) n -> c p n", p=P)
    wq_re = wq_ext.ap().rearrange("(c p) e -> c p e", p=P)
    wk_re = wk_ext.ap().rearrange("(c p) e -> c p e", p=P)
    wv_re = wv_ext.ap().rearrange("(c p) e -> c p e", p=P)
    out_re = out_ext.ap().rearrange("(c p) m -> c p m", p=P)

    with tile.TileContext(nc) as tc:
        with (
            tc.tile_pool(name="const", bufs=1) as const,
            tc.tile_pool(name="stage", bufs=3) as stage,
            tc.tile_pool(name="big", bufs=21) as big,
            tc.tile_pool(name="small", bufs=4) as small,
            tc.tile_pool(name="dram", bufs=1, space="DRAM") as dram,
            tc.tile_pool(name="psum", bufs=4, space="PSUM") as psum,
        ):
            recip_z = const.tile([P, CN], F32, tag="rz")

            # All big tensors share one 8KB/partition slot tag so SBUF slots
            # recycle across phases (peak ~20 live of 21 slots).
            wq_t = [big.tile([P, 4, 1024], BF16, tag="big", name=f"wq{i}") for i in range(2)]
            wk_t = [big.tile([P, 4, 1024], BF16, tag="big", name=f"wk{i}") for i in range(2)]
            wvt_t = [big.tile([P, 4, 1024], BF16, tag="big", name=f"wvt{i}") for i in range(2)]
            gt_t = [big.tile([P, 4, 1024], BF16, tag="big", name=f"gt{i}") for i in range(2)]
            x_t = [big.tile([P, 2, N], BF16, tag="big", name=f"x{i}") for i in range(4)]
            u_t = [big.tile([P, 2, N], BF16, tag="big", name=f"u{i}") for i in range(4)]
            vt_t = [big.tile([P, 4, 1024], BF16, tag="big", name=f"vt{i}") for i in range(4)]
            attn_t = [big.tile([P, 2, N], BF16, tag="big", name=f"at{i}") for i in range(8)]

            _ps_n = [0]

            def ps_tile():
                _ps_n[0] += 1
                return psum.tile(
                    [P, 512], F32, tag="ps", bufs=8, name=f"ps{_ps_n[0]}"
                )



            # ---- load + cast W_q, W_k (interleaved so GT starts early) -----
            for c in range(CE):
                for w_re_, w_t in ((wq_re, wq_t), (wk_re, wk_t)):
                    st = stage.tile([P, D], F32, tag="stage")
                    nc.sync.dma_start(st[:], w_re_[c])
                    nc.vector.tensor_copy(w_t[c // 4][:, c % 4, :], st[:])

            # ---- load + cast x [e, n] --------------------------------------
            # Casts on the (otherwise idle) scalar engine: the in-order
            # vector engine must not head-of-line block the GT/U psum evicts
            # behind these DMA-gated casts.
            # x rides the scalar-engine HWDGE queue, concurrent with the
            # weight loads on the sync queue.
            for c in range(CE):
                st = stage.tile([P, N], F32, tag="stage")
                nc.scalar.dma_start(st[:], x_re[c])
                nc.scalar.copy(x_t[c // 2][:, c % 2, :], st[:])

            # ---- W_v: load, cast, XBAR-transpose via DRAM scratch ----------
            wv_scratch = dram.tile([D, D], BF16)
            for cd in range(CE):
                st = stage.tile([P, D], F32, tag="stage")
                nc.sync.dma_start(st[:], wv_re[cd])
                wvc = small.tile([P, D], BF16, tag="wvc", bufs=2)
                nc.scalar.copy(wvc[:], st[:])
                nc.sync.dma_start(wv_scratch[cd * P:(cd + 1) * P, :], wvc[:])
            for ce in range(CE):
                nc.sync.dma_start(
                    out=wvt_t[ce // 4][:, ce % 4, :],
                    in_=wv_scratch[:, ce * P:(ce + 1) * P],
                    transpose=True,
                )

            # ---- GT = W_k^T W_q  [f, e] ------------------------------------
            # Waves of 8 single-bank psum tiles with the contraction (dc)
            # loop outermost: each arriving W chunk pair feeds 8 issuable
            # matmuls, and single-bank tiles recycle finely at wave edges.
            for wave in range(2):
                cfs = range(wave * 4, wave * 4 + 4)
                tiles = {(cf, et): ps_tile() for cf in cfs for et in range(2)}
                for dc in range(CE):
                    for cf in cfs:
                        for et in range(2):
                            nc.tensor.matmul(
                                tiles[cf, et][:],
                                wk_t[dc // 4][:, dc % 4, cf * P:(cf + 1) * P],
                                wq_t[dc // 4][:, dc % 4, et * 512:(et + 1) * 512],
                                start=(dc == 0),
                                stop=(dc == CE - 1),
                            )
                for (cf, et), ps in tiles.items():
                    nc.vector.tensor_copy(
                        gt_t[cf // 4][:, cf % 4, et * 512:(et + 1) * 512], ps[:]
                    )

            # ---- U = GT^T x = (W_q^T W_k) x  [e, m] ------------------------
            for wave in range(4):
                ces = (2 * wave, 2 * wave + 1)
                tiles = {(ce, mt): ps_tile() for ce in ces for mt in range(4)}
                for cf in range(CE):
                    for ce in ces:
                        for mt in range(4):
                            nc.tensor.matmul(
                                tiles[ce, mt][:],
                                gt_t[cf // 4][:, cf % 4, ce * P:(ce + 1) * P],
                                x_t[cf // 2][:, cf % 2, mt * 512:(mt + 1) * 512],
                                start=(cf == 0),
                                stop=(cf == CE - 1),
                            )
                for (ce, mt), ps in tiles.items():
                    nc.vector.tensor_copy(
                        u_t[ce // 2][:, ce % 2, mt * 512:(mt + 1) * 512], ps[:]
                    )

            # ---- scores + softmax per 128-row chunk ------------------------
            for cn in range(CN):
                quads = [ps_tile() for _ in range(4)]
                for ce in range(CE):
                    for q in range(4):
                        nc.tensor.matmul(
                            quads[q][:],
                            x_t[ce // 2][:, ce % 2, cn * P:(cn + 1) * P],
                            u_t[ce // 2][:, ce % 2, q * 512:(q + 1) * 512],
                            start=(ce == 0),
                            stop=(ce == CE - 1),
                        )
                mx = [small.tile([P, 1], F32, tag="sm", bufs=8, name=f"mx{cn}_{q}") for q in range(4)]
                for q in range(4):
                    nc.vector.reduce_max(mx[q][:], quads[q][:], axis=mybir.AxisListType.X)
                bias = small.tile([P, 1], F32, tag="sm", bufs=8)
                nc.vector.tensor_max(mx[0][:], mx[0][:], mx[1][:])
                nc.vector.tensor_max(mx[2][:], mx[2][:], mx[3][:])
                nc.vector.tensor_max(bias[:], mx[0][:], mx[2][:])
                nc.vector.tensor_scalar_mul(bias[:], bias[:], -K_SCALE)
                zq = [small.tile([P, 1], F32, tag="sm", bufs=8, name=f"z{cn}_{q}") for q in range(4)]
                for q in range(4):
                    nc.scalar.activation(
                        attn_t[cn // 2][:, cn % 2, q * 512:(q + 1) * 512],
                        quads[q][:],
                        mybir.ActivationFunctionType.Exp,
                        bias=bias[:],
                        scale=K_SCALE,
                        accum_out=zq[q][:],
                    )
                nc.vector.tensor_add(zq[0][:], zq[0][:], zq[1][:])
                nc.vector.tensor_add(zq[2][:], zq[2][:], zq[3][:])
                nc.vector.tensor_add(zq[0][:], zq[0][:], zq[2][:])
                nc.vector.reciprocal(recip_z[:, cn:cn + 1], zq[0][:])

            # ---- VT = x^T WvT  [n, d] --------------------------------------
            # After scores in the PE stream: WvT's DMA transpose arrives late
            # in the load queue, and nothing before AV needs VT.
            for cn in range(CN):
                vts = [ps_tile() for _ in range(2)]
                for ce in range(CE):
                    for dt in range(2):
                        nc.tensor.matmul(
                            vts[dt][:],
                            x_t[ce // 2][:, ce % 2, cn * P:(cn + 1) * P],
                            wvt_t[ce // 4][:, ce % 4, dt * 512:(dt + 1) * 512],
                            start=(ce == 0),
                            stop=(ce == CE - 1),
                        )
                for dt in range(2):
                    nc.vector.tensor_copy(
                        vt_t[cn // 4][:, cn % 4, dt * 512:(dt + 1) * 512], vts[dt][:]
                    )

            # ---- fold 1/Z into VT rows -------------------------------------
            for cn in range(CN):
                nc.vector.tensor_scalar_mul(
                    vt_t[cn // 4][:, cn % 4, :],
                    vt_t[cn // 4][:, cn % 4, :],
                    recip_z[:, cn:cn + 1],
                )

            # ---- out = VTs^T attn  [d, m] ----------------------------------
            for dt in range(CE):
                ot = stage.tile([P, N], F32, tag="stage")
                outs = [ps_tile() for _ in range(4)]
                for cn in range(CN):
                    for q in range(4):
                        nc.tensor.matmul(
                            outs[q][:],
                            vt_t[cn // 4][:, cn % 4, dt * P:(dt + 1) * P],
                            attn_t[cn // 2][:, cn % 2, q * 512:(q + 1) * 512],
                            start=(cn == 0),
                            stop=(cn == CN - 1),
                        )
                for q in range(4):
                    nc.vector.tensor_copy(ot[:, q * 512:(q + 1) * 512], outs[q][:])
                    if q % 2 == 1:
                        nc.sync.dma_start(
                            out_re[dt][:, (q - 1) * 512:(q + 1) * 512],
                            ot[:, (q - 1) * 512:(q + 1) * 512],
                        )

    nc.compile()
    return nc


_NC = None


def _get_nc():
    global _NC
    if _NC is None:
        _NC = build_nc()
    return _NC


def make_in_maps(x, W_q, W_k, W_v):
    return [
        {"x": x[i], "W_q": W_q, "W_k": W_k, "W_v": W_v} for i in range(B)
    ]


def kernel(x, W_q, W_k, W_v):
    x = np.ascontiguousarray(np.asarray(x, dtype=np.float32))
    W_q = np.ascontiguousarray(np.asarray(W_q, dtype=np.float32))
    W_k = np.ascontiguousarray(np.asarray(W_k, dtype=np.float32))
    W_v = np.ascontiguousarray(np.asarray(W_v, dtype=np.float32))
    assert x.shape == (B, D, N), x.shape

    nc = _get_nc()
    res = run_bass_kernel_spmd(
        nc, make_in_maps(x, W_q, W_k, W_v), core_ids=list(range(B))
    )
    return np.stack([res.results[i]["out"] for i in range(B)], axis=0)


if __name__ == "__main__":
    rng = np.random.default_rng(0)
    scale = 1.0 / np.sqrt(D)
    x = rng.standard_normal((B, D, N), dtype=np.float32)
    wq = rng.standard_normal((D, D), dtype=np.float32) * scale
    wk = rng.standard_normal((D, D), dtype=np.float32) * scale
    wv = rng.standard_normal((D, D), dtype=np.float32) * scale
    out = kernel(x, wq, wk, wv)
    print("out", out.shape, out.dtype, np.abs(out).max())
